# revision 24
# baseline (speedup 1.0000x reference)
"""Trainium2 Bass kernel for nn_BinaryLinear: out = sign(x @ sign(W).T + bias).

Strategy
--------
Data-parallel over the 8192-token dim: each of the 8 cores gets 1024 tokens
and the full weight matrix.

On-chip compute (per core) is the NT GEMM z.T = sign(W) @ x.T on the
TensorEngine with the contraction (in_features) on the partition dim:

  psum[outf, tok] = sum_k w_b_T[k, outf] * x_T[k, tok]

Both operands are pre-transposed on the host (pure layout prep) so every DMA
is contiguous-per-partition. Precision/speed: x is split as

  x ~= fp16(x) + 2^-6 * e4m3((x - fp16(x)) * 2^6)        (~15-16 mantissa bits)

The hi half runs as regular fp16 matmuls (1 PE cycle/row). The lo half runs
as fp8e4m3 DoubleRow matmuls (0.5 cycles/row, 256-deep contraction per MM)
with the 2^-6 scale folded into the fp8 weights (+-2^-6 is exact in e4m3),
so BOTH halves accumulate into the same fp32 PSUM group with no epilogue
combine. Combined error lands at the fp32 reference's own accumulation-error
scale. fp32 matmul would be 4 cycles/row; a bf16 hi+lo split is 2 cycles/row;
this scheme is 1.5 cycles/row.

sign(W) is computed on-chip (ScalarE Sign: fp32 -> fp16 +-1, then VectorE
*2^-6 -> e4m3). Epilogue fuses bias-add + sign + PSUM->SBUF in one ScalarE
activation (bias is per-partition in the z.T layout). Output is written as
z.T [out_features, tokens] per core and untransposed on the host.

The first four (W-block, token-block) iterations are ordered
(0,n0),(1,n0),(0,n1),(1,n1) so early matmuls only depend on the first half
of the streamed-in resident x, hiding the initial x load entirely.
"""

import numpy as np

import concourse.tile as tile
import concourse.mybir as mybir
from concourse import bacc
from concourse.bass_utils import run_bass_kernel_spmd
from concourse.tile_rust import add_dep_helper

N_CORES = 8
N_TOK = 8192
D_IN = 4096
D_OUT = 4096
P = 128
T = N_TOK // N_CORES  # 1024 tokens per core
KT = D_IN // P  # 32 contraction tiles
KP = KT // 2  # 16 DoubleRow k-pairs
MT = D_OUT // P  # 32 out-feature tiles
M2 = 2  # m-tiles per cached W block (256 outf cols)
MB = MT // M2  # 16 W blocks
TB = 512  # token block (one PSUM bank of fp32)
NB = T // TB  # 2 token blocks per core
LO_SCALE = 2.0 ** 6  # host-side scale on the fp8 residual; inverse on weights

F32 = mybir.dt.float32
FP16 = mybir.dt.float16
FP8 = mybir.dt.float8e4
SIGN = mybir.ActivationFunctionType.Sign
DR = mybir.MatmulPerfMode.DoubleRow
E4M3 = mybir.dt.np(FP8)

_nc_cache = None


def build():
    """Build + compile the per-core Bass/Tile module (SPMD: same on all cores)."""
    global _nc_cache
    if _nc_cache is not None:
        return _nc_cache
    nc = bacc.Bacc("TRN2", target_bir_lowering=False, debug=False, num_devices=N_CORES)
    # x shards arrive token-block-major then partition-major
    # ([n-block, P, k-tiles, TB]) so each resident DMA is one fat contiguous
    # line per partition (128 descriptors) — DMA issue time is
    # per-descriptor — and the n=0 half can stream in entirely before the
    # n=1 half, matching first-iteration consumption to HBM bandwidth.
    xhi_d = nc.dram_tensor("x_hi_t", [NB, P, KT, TB], FP16, kind="ExternalInput").ap()
    xlo_d = nc.dram_tensor(
        "x_lo8_t", [NB, P, KP, 2, TB], FP8, kind="ExternalInput"
    ).ap()
    w_d = nc.dram_tensor("w_t", [D_IN, D_OUT], F32, kind="ExternalInput").ap()
    b_d = nc.dram_tensor("bias", [D_OUT], F32, kind="ExternalInput").ap()
    out_d = nc.dram_tensor("out_t", [D_OUT, T], F32, kind="ExternalOutput").ap()

    with tile.TileContext(nc) as tc:
        with (
            tc.tile_pool(name="x", bufs=1) as x_pool,
            tc.tile_pool(name="wstage", bufs=4) as wstage_pool,
            tc.tile_pool(name="wsb", bufs=3) as w_pool,
            tc.tile_pool(name="bias", bufs=1) as b_pool,
            tc.tile_pool(name="out", bufs=4) as out_pool,
            tc.tile_pool(name="psum", bufs=8, space="PSUM") as psum_pool,
        ):
            def convert_w_block(mb):
                # Stage a [D_IN, 256] W column block; convert to
                # sign() in fp16 (+-1) and e4m3 (+-2^-6).
                wsb_hi = w_pool.tile([P, KT, M2 * P], FP16, tag="wsb_hi",
                                     name=f"wsb_hi_{mb}")
                wsb_lo = w_pool.tile([P, KT, M2 * P], FP8, tag="wsb_lo",
                                     name=f"wsb_lo_{mb}")
                for k in range(KT):
                    wstage = wstage_pool.tile([P, M2 * P], F32, tag="wstage",
                                              name=f"wstage_{mb}_{k}")
                    nc.sync.dma_start(
                        wstage[:],
                        w_d[k * P : (k + 1) * P, mb * M2 * P : (mb + 1) * M2 * P],
                    )
                    nc.scalar.activation(wsb_hi[:, k, :], wstage[:], SIGN)
                    nc.vector.tensor_scalar_mul(
                        wsb_lo[:, k, :], wsb_hi[:, k, :], 1.0 / LO_SCALE
                    )
                return wsb_hi, wsb_lo

            # mb0's W conversion is emitted first so its ScalarE/VectorE ops
            # run ahead of the x-lo DMA stream on the vector queue.
            wsb_cache = {0: convert_w_block(0)}

            # Resident x in per-group tiles (4 hi k-tiles / 2 lo k-pairs per
            # DMA) on the gpsimd queue; the sync queue streams W. hi groups
            # first in k order, then lo groups: within each PSUM group all
            # fp16 MMs run before the DoubleRow MMs, so the lo data is needed
            # one hi-phase later and the DMA stream stays ahead of the PE.
            # Small groups first so early k-tiles complete quickly; n=0
            # before n=1; the lo stream issues on the scalar queue after
            # mb0's sign ops, giving the hi stream the early HBM bandwidth.
            hi_sizes = [1, 1, 2, 4, 8, 8, 8]  # k-tiles per hi group
            lo_sizes = [1, 1, 2, 4, 8]  # k-pairs per lo group
            xhi_map = {}  # (k, n) -> (tile, idx)
            xlo_map = {}  # (t, n) -> (tile, idx)
            deferred_x_dmas = []  # n=1 loads, gated on early mb0/n0 compute
            for n in range(NB):
                off = 0
                for g, sz in enumerate(hi_sizes):
                    th = x_pool.tile([P, sz, TB], FP16, tag=f"xh_{n}_{g}",
                                     name=f"xh_{n}_{g}")
                    dma = nc.gpsimd.dma_start(th[:], xhi_d[n][:, off : off + sz, :])
                    if n > 0:
                        deferred_x_dmas.append(dma.ins)
                    for i in range(sz):
                        xhi_map[(off + i, n)] = (th, i)
                    off += sz
            for n in range(NB):
                off = 0
                for g, sz in enumerate(lo_sizes):
                    tl = x_pool.tile([P, sz, 2, TB], FP8, tag=f"xl_{n}_{g}",
                                     name=f"xl_{n}_{g}")
                    dma = nc.scalar.dma_start(tl[:], xlo_d[n][:, off : off + sz, :, :])
                    if n > 0:
                        deferred_x_dmas.append(dma.ins)
                    for i in range(sz):
                        xlo_map[(off + i, n)] = (tl, i)
                    off += sz
            gate_mm = None  # set at mb0/n0 k=15; n=1 x loads wait on it
            # bias, outf-partition-major: bias_sb[p, mo] = bias[mo*128 + p]
            bias_sb = b_pool.tile([P, MT], F32, tag="bias")
            nc.sync.dma_start(bias_sb[:], b_d.rearrange("(mo p) -> p mo", p=P))

            for mb in range(MB):
                if mb not in wsb_cache:
                    wsb_cache[mb] = convert_w_block(mb)
                wsb_hi, wsb_lo = wsb_cache.pop(mb)

                for n in range(NB):
                    nsl = slice(n * TB, (n + 1) * TB)
                    psums = [
                        psum_pool.tile([P, TB], F32, tag="psum",
                                       name=f"ps_{mb}_{n}_{mi}")
                        for mi in range(M2)
                    ]
                    for k in range(KT):
                        xt, xi = xhi_map[(k, n)]
                        for mi in range(M2):
                            msl = slice(mi * P, (mi + 1) * P)
                            mm = nc.tensor.matmul(
                                psums[mi][:],
                                wsb_hi[:, k, msl],
                                xt[:, xi, :],
                                start=(k == 0),
                                stop=False,
                            )
                            if mb == 0 and n == 0 and k == 15 and mi == M2 - 1:
                                gate_mm = mm.ins
                    for t in range(KP):
                        lt, li = xlo_map[(t, n)]
                        for mi in range(M2):
                            msl = slice(mi * P, (mi + 1) * P)
                            nc.tensor.matmul(
                                psums[mi][:],
                                wsb_lo[:, 2 * t : 2 * t + 2, msl],
                                lt[:, li, :, :],
                                start=False,
                                stop=(t == KP - 1),
                                perf_mode=DR,
                            )
                    for mi in range(M2):
                        m = mb * M2 + mi
                        osb = out_pool.tile([P, TB], F32, tag="osb",
                                            name=f"osb_{mb}_{n}_{mi}")
                        nc.scalar.activation(
                            osb[:], psums[mi][:], SIGN, bias=bias_sb[:, m : m + 1]
                        )
                        nc.sync.dma_start(
                            out_d[m * P : (m + 1) * P, nsl], osb[:]
                        )
            if gate_mm is not None:
                for dma_ins in deferred_x_dmas:
                    add_dep_helper(
                        dma_ins,
                        gate_mm,
                        sync=True,
                        reason="defer n=1 x load until early n=0 compute",
                    )
    nc.compile()
    _nc_cache = nc
    return nc


def prep_in_maps(x, weight, bias):
    """Host-side layout prep: fp16/fp8 split of x, transposes, token shards."""
    x = np.asarray(x, dtype=np.float32)
    weight = np.asarray(weight, dtype=np.float32)
    bias = np.asarray(bias, dtype=np.float32)

    x_hi = x.astype(np.float16)
    x_lo8 = ((x - x_hi.astype(np.float32)) * LO_SCALE).astype(E4M3)
    # Token-block- and partition-major resident layouts (see build()):
    #   hi[n, p, ko, tb] = x.T[ko*128 + p, n*TB + tb]   (per-core tokens)
    #   lo[n, p, t2, j, tb] = x.T[t2*256 + j*128 + p, n*TB + tb]
    xhi_t = x_hi.T  # [D_IN, N_TOK]
    xlo_t = x_lo8.T
    w_t = np.ascontiguousarray(weight.T)  # [D_IN, D_OUT]

    in_maps = []
    for c in range(N_CORES):
        sl = slice(c * T, (c + 1) * T)
        xh = np.ascontiguousarray(
            xhi_t[:, sl].reshape(KT, P, NB, TB).transpose(2, 1, 0, 3)
        )  # [NB, P, KT, TB]
        xl = np.ascontiguousarray(
            xlo_t[:, sl].reshape(KP, 2, P, NB, TB).transpose(3, 2, 0, 1, 4)
        )  # [NB, P, KP, 2, TB]
        in_maps.append(
            {
                "x_hi_t": xh,
                "x_lo8_t": xl,
                "w_t": w_t,
                "bias": bias,
            }
        )
    return in_maps


def run(x, weight, bias, **spmd_kwargs):
    """Run on the 8 cores; returns (full_output, BassKernelResults)."""
    nc = build()
    in_maps = prep_in_maps(x, weight, bias)
    res = run_bass_kernel_spmd(nc, in_maps, core_ids=list(range(N_CORES)), **spmd_kwargs)
    out = np.empty((N_TOK, D_OUT), dtype=np.float32)
    for c in range(N_CORES):
        out[c * T : (c + 1) * T, :] = res.results[c]["out_t"].T
    return out, res


def kernel(x, weight, bias):
    out, _ = run(x, weight, bias)
    return out
